# revision 1
# baseline (speedup 1.0000x reference)
"""Trainium2 Bass kernel for ItemEmbeddingLayer (embedding_lookup).

Reference computation:
    out = Q_matrix[items] @ skill_embedding[user]      # [8192, 128] f32

Sharding (per the hint): the single active user's embedding row
(skill_embedding[user], [256,128]) is replicated to all 8 cores; `items`
is sharded batch-wise, 1024 per core; Q_matrix is replicated (each core
gathers only the rows its items need via indirect DMA).

Per-core device kernel:
  1. 8x indirect_dma_start gathers pull the 1024 needed Q rows (bf16 —
     exact, Q is binary) into [item, skill] SBUF tiles, 1 row/partition.
  2. PE transposes (with a bf16 identity) flip each [128,128] block into
     the [skill, item] layout matmul weights need; DVE/ACT copy back.
  3. skill_embedding row is split on-device into bf16 hi + lo parts
     (emb ~= hi + lo), recovering ~fp32 precision from bf16 matmuls.
  4. 8 l-chunks x (2 s-chunks x {hi,lo}) matmuls accumulate in fp32 PSUM.
  5. PSUM -> SBUF copies, one 512KB DMA out.
"""

import numpy as np
import ml_dtypes

import concourse.bass as bass
import concourse.bacc as bacc
import concourse.mybir as mybir
from concourse.tile import TileContext
from concourse.bass_utils import run_bass_kernel_spmd

N_CORES = 8
L = 8192          # total items (seq len)
LC = L // N_CORES # items per core
S = 256           # skills
K = 128           # hidden
R = 4096          # Q_matrix rows (n items vocab)
P = 128           # partitions
NCH = LC // P     # l-chunks per core


def build_bass() -> bass.Bass:
    nc = bacc.Bacc(trn_type="TRN2", dynamic_dma_scratch_size=131072)
    q = nc.declare_dram_parameter("q_bf16", [R, S], mybir.dt.bfloat16, isOutput=False)
    idx = nc.declare_dram_parameter("idx", [P, NCH], mybir.dt.int32, isOutput=False)
    emb = nc.declare_dram_parameter("emb", [S, K], mybir.dt.float32, isOutput=False)
    ident = nc.declare_dram_parameter("ident", [P, P], mybir.dt.bfloat16, isOutput=False)
    out = nc.declare_dram_parameter("out", [LC, K], mybir.dt.float32, isOutput=True)

    with (
        TileContext(nc) as tc,
        tc.tile_pool(name="main", bufs=1) as pool,
        tc.tile_pool(name="gat", bufs=4) as gpool,
        tc.tile_pool(name="tps", bufs=4, space="PSUM") as tpsum,
        tc.tile_pool(name="acc", bufs=4, space="PSUM") as apsum,
    ):
        idx_t = pool.tile([P, NCH], mybir.dt.int32)
        nc.sync.dma_start(out=idx_t[:], in_=idx[:])
        ident_t = pool.tile([P, P], mybir.dt.bfloat16)
        nc.sync.dma_start(out=ident_t[:], in_=ident[:])

        emb_t = pool.tile([P, 2, K], mybir.dt.float32)
        nc.sync.dma_start(out=emb_t[:], in_=emb[:].rearrange("(e p) k -> p e k", p=P))

        # emb = hi + lo with both parts bf16; products accumulate in fp32
        # PSUM, so two bf16 passes recover ~16 mantissa bits of emb.
        # hilo[:, e, :] = [hi_e | lo_e] so one N=256 matmul does both passes.
        hilo = pool.tile([P, 2, 2 * K], mybir.dt.bfloat16)
        nc.vector.tensor_copy(hilo[:, :, 0:K], emb_t[:])
        hi32 = pool.tile([P, 2, K], mybir.dt.float32)
        nc.vector.tensor_copy(hi32[:], hilo[:, :, 0:K])
        nc.vector.tensor_sub(hilo[:, :, K : 2 * K], emb_t[:], hi32[:])

        for c in range(NCH):
            # q_sb[p, s] = Q[idx[p, c], s] = Q[items[c*128 + p], s]
            q_sb = gpool.tile([P, S], mybir.dt.bfloat16, tag="q_sb")
            nc.gpsimd.indirect_dma_start(
                out=q_sb[:],
                out_offset=None,
                in_=q[:],
                in_offset=bass.IndirectOffsetOnAxis(ap=idx_t[:, c : c + 1], axis=0),
            )
            qT = gpool.tile([P, 2, P], mybir.dt.bfloat16, tag="qT")
            for e in range(2):
                tp = tpsum.tile([P, P], mybir.dt.bfloat16, tag="tp")
                nc.tensor.transpose(
                    out=tp[:], in_=q_sb[:, e * P : (e + 1) * P], identity=ident_t[:]
                )
                # alternate copy engine so DVE and ACT share the load
                if e == 0:
                    nc.vector.tensor_copy(qT[:, e, :], tp[:])
                else:
                    nc.scalar.copy(qT[:, e, :], tp[:])

            # ps[:, :K] = q@hi, ps[:, K:] = q@lo (e-sum via PSUM accumulate)
            ps = apsum.tile([P, 2 * K], mybir.dt.float32, tag="ps")
            for e in range(2):
                nc.tensor.matmul(
                    ps[:], qT[:, e, :], hilo[:, e, :],
                    start=(e == 0), stop=(e == 1),
                )
            o = gpool.tile([P, K], mybir.dt.float32, tag="o")
            nc.scalar.copy(o[:], ps[:, 0:K])
            nc.vector.tensor_add(o[:], o[:], ps[:, K : 2 * K])
            nc.sync.dma_start(out=out[c * P : (c + 1) * P, :], in_=o[:])

    nc.compile()
    return nc


_CACHE: dict = {}


def get_nc() -> bass.Bass:
    if "nc" not in _CACHE:
        _CACHE["nc"] = build_bass()
    return _CACHE["nc"]


def make_in_maps(user, Q_matrix, items, skill_embedding):
    user = int(np.asarray(user))
    Q = np.asarray(Q_matrix, dtype=np.float32)
    items = np.asarray(items).astype(np.int64)
    emb = np.ascontiguousarray(np.asarray(skill_embedding)[user], dtype=np.float32)
    q_bf = Q.astype(ml_dtypes.bfloat16)  # exact: Q is 0/1
    ident = np.eye(P, dtype=ml_dtypes.bfloat16)

    in_maps = []
    for i in range(N_CORES):
        it = items[i * LC : (i + 1) * LC].astype(np.int32)
        # indirect gather c pulls row idx[p, c] into partition p
        idx_arr = np.ascontiguousarray(it.reshape(NCH, P).T)  # [128, NCH]
        in_maps.append({"q_bf16": q_bf, "idx": idx_arr, "emb": emb, "ident": ident})
    return in_maps


def kernel(user, Q_matrix, items, skill_embedding, _trace=False, _result_box=None):
    in_maps = make_in_maps(user, Q_matrix, items, skill_embedding)
    res = run_bass_kernel_spmd(get_nc(), in_maps, list(range(N_CORES)), trace=_trace)
    if _result_box is not None:
        _result_box.append(res)
    out = np.concatenate([res.results[i]["out"] for i in range(N_CORES)], axis=0)
    return np.ascontiguousarray(out, dtype=np.float32)



# revision 2
# speedup vs baseline: 1.1099x; 1.1099x over previous
"""Trainium2 Bass kernel for ItemEmbeddingLayer (embedding_lookup).

Reference computation:
    out = Q_matrix[items] @ skill_embedding[user]      # [8192, 128] f32

Sharding: items split 1024/core across 8 cores (data parallel); Q (bf16,
exact - Q is binary) and the single user's embedding row (bf16) replicated.

Per-core device kernel (computes out' = out^T; host transposes back):
  1. 8x indirect_dma_start gathers (128 rows each, the SWDGE per-instruction
     fixed cost makes this the pacing chain) -> q_sb[j] [128(item), 256(skill)]
  2. PE transposes each [128,128] block into [skill, item] layout (qT),
     DVE copies PSUM->SBUF, pipelined per chunk behind the gathers.
  3. Matmuls with the embedding as stationary weights:
     ps[k, l] += emb[s,c,k]^T . qT[s,c,l], both skill-chunks accumulated
     in fp32 PSUM. Output regions split 512/256/128/128 wide so the last
     gathered chunk has minimal dependent work (short tail).
  4. DVE copies PSUM->SBUF as bf16, 2D DMA out per region (sync/scalar).
Host: concat per-core [128, 1024] -> [128, 8192] -> transpose -> [8192, 128].
"""

import numpy as np
import ml_dtypes

import concourse.bass as bass
import concourse.bacc as bacc
import concourse.mybir as mybir
from concourse.tile import TileContext
from concourse.bass_utils import run_bass_kernel_spmd

N_CORES = 8
L = 8192
LC = L // N_CORES
S = 256
K = 128
R = 4096
P = 128
NCH = LC // P

PSUM_DMA = False    # DMA outputs straight from PSUM (no SBUF copy)
HILO = False       # single bf16 E (False) vs hi+lo split (True)

# (start_chunk, end_chunk) per output region; region width = 128*(e-s)
REGIONS = [(0, 4), (4, 6), (6, 7), (7, 8)]


def build_bass() -> bass.Bass:
    nc = bacc.Bacc(trn_type="TRN2", dynamic_dma_scratch_size=131072)
    q = nc.declare_dram_parameter("q_bf16", [R, S], mybir.dt.bfloat16, isOutput=False)
    idx = nc.declare_dram_parameter("idx", [P, NCH], mybir.dt.int32, isOutput=False)
    nE = 2 if HILO else 1
    emb = nc.declare_dram_parameter("emb", [P, 2, nE * K], mybir.dt.bfloat16, isOutput=False)
    ident = nc.declare_dram_parameter("ident", [P, P], mybir.dt.bfloat16, isOutput=False)
    out = nc.declare_dram_parameter("out", [K, LC], mybir.dt.bfloat16, isOutput=True)

    with (
        TileContext(nc) as tc,
        tc.tile_pool(name="main", bufs=1) as pool,
        tc.tile_pool(name="gat", bufs=NCH) as gpool,
        tc.tile_pool(name="tps", bufs=4, space="PSUM") as tpsum,
        tc.tile_pool(name="acc", bufs=1, space="PSUM") as apsum,
    ):
        idx_t = pool.tile([P, NCH], mybir.dt.int32)
        nc.sync.dma_start(out=idx_t[:], in_=idx[:])
        emb_t = pool.tile([P, 2, nE * K], mybir.dt.bfloat16)
        nc.scalar.dma_start(out=emb_t[:], in_=emb[:])
        ident_t = pool.tile([P, P], mybir.dt.bfloat16)
        nc.scalar.dma_start(out=ident_t[:], in_=ident[:])

        qT = pool.tile([P, 2, LC], mybir.dt.bfloat16)

        def mm_region(r):
            s0, s1 = REGIONS[r]
            n = (s1 - s0) * P
            ps = apsum.tile([P, n], mybir.dt.float32, tag=f"ps{r}")
            first = True
            for c in range(2):
                for e in range(nE):
                    nc.tensor.matmul(
                        ps[:],
                        emb_t[:, c, e * K : (e + 1) * K],
                        qT[:, c, s0 * P : s1 * P],
                        start=first,
                        stop=(c == 1 and e == nE - 1),
                    )
                    first = False
            engs = [nc.sync, nc.scalar, nc.scalar, nc.sync]
            if PSUM_DMA:
                engs[r].dma_start(out=out[:, s0 * P : s1 * P], in_=ps[:])
            else:
                o = pool.tile([P, n], mybir.dt.bfloat16, tag=f"o{r}")
                nc.vector.tensor_copy(o[:], ps[:])
                engs[r].dma_start(out=out[:, s0 * P : s1 * P], in_=o[:])

        region_of_chunk = {}
        for r, (s0, s1) in enumerate(REGIONS):
            region_of_chunk[s1 - 1] = r

        for j in range(NCH):
            t = gpool.tile([P, S], mybir.dt.bfloat16, tag=f"q{j}")
            nc.gpsimd.indirect_dma_start(
                out=t[:],
                out_offset=None,
                in_=q[:],
                in_offset=bass.IndirectOffsetOnAxis(ap=idx_t[:, j : j + 1], axis=0),
            )
            for c in range(2):
                tp = tpsum.tile([P, P], mybir.dt.bfloat16, tag="tp")
                nc.tensor.transpose(
                    out=tp[:], in_=t[:, c * P : (c + 1) * P], identity=ident_t[:]
                )
                nc.vector.tensor_copy(qT[:, c, j * P : (j + 1) * P], tp[:])
            if j in region_of_chunk:
                mm_region(region_of_chunk[j])

    nc.compile()
    return nc


_CACHE: dict = {}


def get_nc() -> bass.Bass:
    if "nc" not in _CACHE:
        _CACHE["nc"] = build_bass()
    return _CACHE["nc"]


def make_in_maps(user, Q_matrix, items, skill_embedding):
    user = int(np.asarray(user))
    Q = np.asarray(Q_matrix, dtype=np.float32)
    items = np.asarray(items).astype(np.int64)
    E = np.ascontiguousarray(np.asarray(skill_embedding)[user], dtype=np.float32)
    q_bf = Q.astype(ml_dtypes.bfloat16)
    ident = np.eye(P, dtype=ml_dtypes.bfloat16)

    nE = 2 if HILO else 1
    hi = E.astype(ml_dtypes.bfloat16)
    emb = np.empty((P, 2, nE * K), dtype=ml_dtypes.bfloat16)
    for c in range(2):
        emb[:, c, 0:K] = hi[c * P : (c + 1) * P, :]
        if HILO:
            lo = (E - hi.astype(np.float32)).astype(ml_dtypes.bfloat16)
            emb[:, c, K : 2 * K] = lo[c * P : (c + 1) * P, :]

    in_maps = []
    for i in range(N_CORES):
        it = items[i * LC : (i + 1) * LC].astype(np.int32)
        idx_arr = np.ascontiguousarray(it.reshape(NCH, P).T)
        in_maps.append({"q_bf16": q_bf, "idx": idx_arr, "emb": emb, "ident": ident})
    return in_maps


def kernel(user, Q_matrix, items, skill_embedding, _trace=False, _result_box=None):
    in_maps = make_in_maps(user, Q_matrix, items, skill_embedding)
    res = run_bass_kernel_spmd(get_nc(), in_maps, list(range(N_CORES)), trace=_trace)
    if _result_box is not None:
        _result_box.append(res)
    full = np.concatenate(
        [np.asarray(res.results[i]["out"]).astype(np.float32) for i in range(N_CORES)],
        axis=1,
    )
    return np.ascontiguousarray(full.T, dtype=np.float32)


# revision 3
# speedup vs baseline: 1.1150x; 1.0046x over previous
"""Trainium2 Bass kernel for ItemEmbeddingLayer (embedding_lookup).

Reference computation:
    out = Q_matrix[items] @ skill_embedding[user]      # [8192, 128] f32

Sharding: items split 1024/core across 8 cores (data parallel); Q (bf16,
exact - Q is binary) and the single user's embedding row (bf16) replicated.

Per-core device kernel (computes out' = out^T; host transposes back):
  1. 8x indirect_dma_start gathers (128 rows each, the SWDGE per-instruction
     fixed cost makes this the pacing chain) -> q_sb[j] [128(item), 256(skill)]
  2. PE transposes each [128,128] block into [skill, item] layout (qT),
     DVE copies PSUM->SBUF, pipelined per chunk behind the gathers.
  3. Matmuls with the embedding as stationary weights:
     ps[k, l] += emb[s,c,k]^T . qT[s,c,l], both skill-chunks accumulated
     in fp32 PSUM. Output regions split 512/256/128/128 wide so the last
     gathered chunk has minimal dependent work (short tail).
  4. DVE copies PSUM->SBUF as bf16, 2D DMA out per region (sync/scalar).
Host: concat per-core [128, 1024] -> [128, 8192] -> transpose -> [8192, 128].
"""

import numpy as np
import ml_dtypes

import concourse.bass as bass
import concourse.bacc as bacc
import concourse.mybir as mybir
from concourse.tile import TileContext
from concourse.bass_utils import run_bass_kernel_spmd

N_CORES = 8
L = 8192
LC = L // N_CORES
S = 256
K = 128
R = 4096
P = 128
NCH = LC // P

PSUM_DMA = False    # DMA outputs straight from PSUM (no SBUF copy)
HILO = False       # single bf16 E (False) vs hi+lo split (True)

# (start_chunk, end_chunk) per output region; region width = 128*(e-s)
REGIONS = [(0, 4), (4, 7), (7, 8)]


def build_bass() -> bass.Bass:
    nc = bacc.Bacc(trn_type="TRN2", dynamic_dma_scratch_size=131072)
    q = nc.declare_dram_parameter("q_bf16", [R, S], mybir.dt.bfloat16, isOutput=False)
    idx = nc.declare_dram_parameter("idx", [P, NCH], mybir.dt.int32, isOutput=False)
    nE = 2 if HILO else 1
    emb = nc.declare_dram_parameter("emb", [P, 2, nE * K], mybir.dt.bfloat16, isOutput=False)
    ident = nc.declare_dram_parameter("ident", [P, P], mybir.dt.bfloat16, isOutput=False)
    out = nc.declare_dram_parameter("out", [K, LC], mybir.dt.bfloat16, isOutput=True)

    with (
        TileContext(nc) as tc,
        tc.tile_pool(name="main", bufs=1) as pool,
        tc.tile_pool(name="gat", bufs=NCH) as gpool,
        tc.tile_pool(name="tps", bufs=4, space="PSUM") as tpsum,
        tc.tile_pool(name="acc", bufs=1, space="PSUM") as apsum,
    ):
        idx_t = pool.tile([P, NCH], mybir.dt.int32)
        nc.sync.dma_start(out=idx_t[:], in_=idx[:])
        emb_t = pool.tile([P, 2, nE * K], mybir.dt.bfloat16)
        nc.scalar.dma_start(out=emb_t[:], in_=emb[:])
        ident_t = pool.tile([P, P], mybir.dt.bfloat16)
        nc.scalar.dma_start(out=ident_t[:], in_=ident[:])

        qT = pool.tile([P, 2, LC], mybir.dt.bfloat16)

        def mm_region(r):
            s0, s1 = REGIONS[r]
            n = (s1 - s0) * P
            ps = apsum.tile([P, n], mybir.dt.float32, tag=f"ps{r}")
            first = True
            for c in range(2):
                for e in range(nE):
                    nc.tensor.matmul(
                        ps[:],
                        emb_t[:, c, e * K : (e + 1) * K],
                        qT[:, c, s0 * P : s1 * P],
                        start=first,
                        stop=(c == 1 and e == nE - 1),
                    )
                    first = False
            engs = [nc.sync, nc.scalar, nc.sync]
            if PSUM_DMA:
                engs[r].dma_start(out=out[:, s0 * P : s1 * P], in_=ps[:])
            else:
                o = pool.tile([P, n], mybir.dt.bfloat16, tag=f"o{r}")
                nc.vector.tensor_copy(o[:], ps[:])
                engs[r].dma_start(out=out[:, s0 * P : s1 * P], in_=o[:])

        region_of_chunk = {}
        for r, (s0, s1) in enumerate(REGIONS):
            region_of_chunk[s1 - 1] = r

        for j in range(NCH):
            t = gpool.tile([P, S], mybir.dt.bfloat16, tag=f"q{j}")
            nc.gpsimd.indirect_dma_start(
                out=t[:],
                out_offset=None,
                in_=q[:],
                in_offset=bass.IndirectOffsetOnAxis(ap=idx_t[:, j : j + 1], axis=0),
            )
            for c in range(2):
                tp = tpsum.tile([P, P], mybir.dt.bfloat16, tag="tp")
                nc.tensor.transpose(
                    out=tp[:], in_=t[:, c * P : (c + 1) * P], identity=ident_t[:]
                )
                nc.vector.tensor_copy(qT[:, c, j * P : (j + 1) * P], tp[:])
            if j in region_of_chunk:
                mm_region(region_of_chunk[j])

    nc.compile()
    return nc


_CACHE: dict = {}


def get_nc() -> bass.Bass:
    if "nc" not in _CACHE:
        _CACHE["nc"] = build_bass()
    return _CACHE["nc"]


def make_in_maps(user, Q_matrix, items, skill_embedding):
    user = int(np.asarray(user))
    Q = np.asarray(Q_matrix, dtype=np.float32)
    items = np.asarray(items).astype(np.int64)
    E = np.ascontiguousarray(np.asarray(skill_embedding)[user], dtype=np.float32)
    q_bf = Q.astype(ml_dtypes.bfloat16)
    ident = np.eye(P, dtype=ml_dtypes.bfloat16)

    nE = 2 if HILO else 1
    hi = E.astype(ml_dtypes.bfloat16)
    emb = np.empty((P, 2, nE * K), dtype=ml_dtypes.bfloat16)
    for c in range(2):
        emb[:, c, 0:K] = hi[c * P : (c + 1) * P, :]
        if HILO:
            lo = (E - hi.astype(np.float32)).astype(ml_dtypes.bfloat16)
            emb[:, c, K : 2 * K] = lo[c * P : (c + 1) * P, :]

    in_maps = []
    for i in range(N_CORES):
        it = items[i * LC : (i + 1) * LC].astype(np.int32)
        idx_arr = np.ascontiguousarray(it.reshape(NCH, P).T)
        in_maps.append({"q_bf16": q_bf, "idx": idx_arr, "emb": emb, "ident": ident})
    return in_maps


def kernel(user, Q_matrix, items, skill_embedding, _trace=False, _result_box=None):
    in_maps = make_in_maps(user, Q_matrix, items, skill_embedding)
    res = run_bass_kernel_spmd(get_nc(), in_maps, list(range(N_CORES)), trace=_trace)
    if _result_box is not None:
        _result_box.append(res)
    full = np.concatenate(
        [np.asarray(res.results[i]["out"]).astype(np.float32) for i in range(N_CORES)],
        axis=1,
    )
    return np.ascontiguousarray(full.T, dtype=np.float32)


# revision 4
# speedup vs baseline: 1.1207x; 1.0051x over previous
"""Trainium2 Bass kernel for ItemEmbeddingLayer (embedding_lookup).

Reference computation:
    out = Q_matrix[items] @ skill_embedding[user]      # [8192, 128] f32

Sharding: items split 1024/core across 8 cores (data parallel); Q (bf16,
exact - Q is binary) and the single user's embedding row (bf16) replicated.

Per-core device kernel (computes out' = out^T; host transposes back):
  1. 8x indirect_dma_start gathers (128 rows each, the SWDGE per-instruction
     fixed cost makes this the pacing chain) -> q_sb[j] [128(item), 256(skill)]
  2. PE transposes each [128,128] block into [skill, item] layout (qT),
     DVE copies PSUM->SBUF, pipelined per chunk behind the gathers.
  3. Matmuls with the embedding as stationary weights:
     ps[k, l] += emb[s,c,k]^T . qT[s,c,l], both skill-chunks accumulated
     in fp32 PSUM. Output regions split 512/256/128/128 wide so the last
     gathered chunk has minimal dependent work (short tail).
  4. DVE copies PSUM->SBUF as bf16, 2D DMA out per region (sync/scalar).
Host: concat per-core [128, 1024] -> [128, 8192] -> transpose -> [8192, 128].
"""

import numpy as np
import ml_dtypes

import concourse.bass as bass
import concourse.bacc as bacc
import concourse.mybir as mybir
from concourse.tile import TileContext
from concourse.bass_utils import run_bass_kernel_spmd

N_CORES = 8
L = 8192
LC = L // N_CORES
S = 256
K = 128
R = 4096
P = 128
NCH = LC // P

PSUM_DMA = False    # DMA outputs straight from PSUM (no SBUF copy)
HILO = False       # single bf16 E (False) vs hi+lo split (True)

# (start_chunk, end_chunk) per output region; region width = 128*(e-s)
REGIONS = [(0, 4), (4, 7), (7, 8)]


def build_bass() -> bass.Bass:
    nc = bacc.Bacc(trn_type="TRN2", dynamic_dma_scratch_size=131072)
    q = nc.declare_dram_parameter("q_bf16", [R, S], mybir.dt.bfloat16, isOutput=False)
    idx = nc.declare_dram_parameter("idx", [P, NCH], mybir.dt.int32, isOutput=False)
    nE = 2 if HILO else 1
    emb = nc.declare_dram_parameter("emb", [P, 2, nE * K], mybir.dt.bfloat16, isOutput=False)
    ident = nc.declare_dram_parameter("ident", [P, P], mybir.dt.bfloat16, isOutput=False)
    out = nc.declare_dram_parameter("out", [K, LC], mybir.dt.bfloat16, isOutput=True)

    with (
        TileContext(nc) as tc,
        tc.tile_pool(name="main", bufs=1) as pool,
        tc.tile_pool(name="gat", bufs=NCH) as gpool,
        tc.tile_pool(name="tps", bufs=4, space="PSUM") as tpsum,
        tc.tile_pool(name="acc", bufs=1, space="PSUM") as apsum,
    ):
        idx_t = pool.tile([P, NCH], mybir.dt.int32)
        nc.sync.dma_start(out=idx_t[:], in_=idx[:])
        emb_t = pool.tile([P, 2, nE * K], mybir.dt.bfloat16)
        nc.scalar.dma_start(out=emb_t[:], in_=emb[:])
        ident_t = pool.tile([P, P], mybir.dt.bfloat16)
        nc.scalar.dma_start(out=ident_t[:], in_=ident[:])

        qT = pool.tile([P, 2, LC], mybir.dt.bfloat16)

        def mm_region(r):
            s0, s1 = REGIONS[r]
            n = (s1 - s0) * P
            ps = apsum.tile([P, n], mybir.dt.float32, tag=f"ps{r}")
            first = True
            for c in range(2):
                for e in range(nE):
                    nc.tensor.matmul(
                        ps[:],
                        emb_t[:, c, e * K : (e + 1) * K],
                        qT[:, c, s0 * P : s1 * P],
                        start=first,
                        stop=(c == 1 and e == nE - 1),
                    )
                    first = False
            engs = [nc.sync, nc.scalar, nc.gpsimd]
            if PSUM_DMA:
                engs[r].dma_start(out=out[:, s0 * P : s1 * P], in_=ps[:])
            else:
                o = pool.tile([P, n], mybir.dt.bfloat16, tag=f"o{r}")
                nc.vector.tensor_copy(o[:], ps[:])
                engs[r].dma_start(out=out[:, s0 * P : s1 * P], in_=o[:])

        region_of_chunk = {}
        for r, (s0, s1) in enumerate(REGIONS):
            region_of_chunk[s1 - 1] = r

        for j in range(NCH):
            t = gpool.tile([P, S], mybir.dt.bfloat16, tag=f"q{j}")
            nc.gpsimd.indirect_dma_start(
                out=t[:],
                out_offset=None,
                in_=q[:],
                in_offset=bass.IndirectOffsetOnAxis(ap=idx_t[:, j : j + 1], axis=0),
            )
            for c in range(2):
                tp = tpsum.tile([P, P], mybir.dt.bfloat16, tag="tp")
                nc.tensor.transpose(
                    out=tp[:], in_=t[:, c * P : (c + 1) * P], identity=ident_t[:]
                )
                nc.vector.tensor_copy(qT[:, c, j * P : (j + 1) * P], tp[:])
            if j in region_of_chunk:
                mm_region(region_of_chunk[j])

    nc.compile()
    return nc


_CACHE: dict = {}


def get_nc() -> bass.Bass:
    if "nc" not in _CACHE:
        _CACHE["nc"] = build_bass()
    return _CACHE["nc"]


def make_in_maps(user, Q_matrix, items, skill_embedding):
    user = int(np.asarray(user))
    Q = np.asarray(Q_matrix, dtype=np.float32)
    items = np.asarray(items).astype(np.int64)
    E = np.ascontiguousarray(np.asarray(skill_embedding)[user], dtype=np.float32)
    q_bf = Q.astype(ml_dtypes.bfloat16)
    ident = np.eye(P, dtype=ml_dtypes.bfloat16)

    nE = 2 if HILO else 1
    hi = E.astype(ml_dtypes.bfloat16)
    emb = np.empty((P, 2, nE * K), dtype=ml_dtypes.bfloat16)
    for c in range(2):
        emb[:, c, 0:K] = hi[c * P : (c + 1) * P, :]
        if HILO:
            lo = (E - hi.astype(np.float32)).astype(ml_dtypes.bfloat16)
            emb[:, c, K : 2 * K] = lo[c * P : (c + 1) * P, :]

    in_maps = []
    for i in range(N_CORES):
        it = items[i * LC : (i + 1) * LC].astype(np.int32)
        idx_arr = np.ascontiguousarray(it.reshape(NCH, P).T)
        in_maps.append({"q_bf16": q_bf, "idx": idx_arr, "emb": emb, "ident": ident})
    return in_maps


def kernel(user, Q_matrix, items, skill_embedding, _trace=False, _result_box=None):
    in_maps = make_in_maps(user, Q_matrix, items, skill_embedding)
    res = run_bass_kernel_spmd(get_nc(), in_maps, list(range(N_CORES)), trace=_trace)
    if _result_box is not None:
        _result_box.append(res)
    full = np.concatenate(
        [np.asarray(res.results[i]["out"]).astype(np.float32) for i in range(N_CORES)],
        axis=1,
    )
    return np.ascontiguousarray(full.T, dtype=np.float32)
